# revision 1
# baseline (speedup 1.0000x reference)
"""Trainium2 Bass kernel for BeansBackbone (ViT-B/16 with sparse Cantor-routed
attention), data-parallel over batch across 8 NeuronCores.

Strategy:
  - One image per core (B=8 over 8 cores), full 12-layer transformer per core.
  - All weights host-folded (LN affine into qkv/fc1, SCALE into Wq), host
    transposed into matmul-ready layouts, and cast to bf16 (memory-bound
    regime: halves HBM traffic; fp32 PSUM accumulation).
  - Sparse KNN attention == dense attention with a per-(q,k) binary mask
    (exp(score) * mask01, then row-normalize). Mask is one [S,S] constant.
  - Device token layout: rows 0..195 = patches, row 196 = CLS (so the patch
    GEMM hits aligned partitions); host un-permutes the output.
  - Residual stream fp32 token-major [197, 768] in SBUF. LN token-major via
    bn_stats. Matmuls: Q/K/fc1 produce feature-major (weights stationary);
    V/proj/fc2/patch produce token-major (activations-transposed stationary).
"""

import numpy as np
import ml_dtypes
from contextlib import ExitStack

import concourse.bass as bass
import concourse.mybir as mybir
import concourse.tile as tile
from concourse.bass_utils import run_bass_kernel_spmd
from concourse.masks import make_identity

F32 = mybir.dt.float32
BF16 = mybir.dt.bfloat16

B = 8
C = 3
IMG = 224
PATCH = 16
G = IMG // PATCH          # 14
P = G * G                 # 196
S = P + 1                 # 197
D = 768
H = 12
HD = D // H               # 64
L = 12
MLP = 4 * D               # 3072
SCALE = HD ** -0.5
N_CORES = 8
KC = D // 128             # 6 contraction chunks of 128
T1, T2 = 128, S - 128     # device token chunks: [0:128), [128:197)
EPS = 1e-5

# token chunks: (q0, qw)
QCH = [(0, T1), (T1, T2)]
SPAD = 198                # even-padded per-head stride (bf16 4B alignment)
# free-dim split of 768-wide outputs into PSUM-bank-sized pieces
NH = [(0, 512), (512, 256)]

_BUILD_CACHE = {}


# ---------------------------------------------------------------- host prep

def _bf16(a):
    return np.asarray(a, np.float32).astype(ml_dtypes.bfloat16)


def _prep_shared(patch_w, patch_b, cls_token, pos_embed, qkv_w, qkv_b,
                 proj_w, proj_b, ln1_s, ln1_b, ln2_s, ln2_b,
                 fc1_w, fc1_b, fc2_w, fc2_b, routes):
    """Fold + transpose + cast all weights into device-ready arrays (numpy)."""
    qkv_w = np.asarray(qkv_w, np.float32)
    fc1_w = np.asarray(fc1_w, np.float32)
    ln1_s = np.asarray(ln1_s, np.float32)
    ln1_b = np.asarray(ln1_b, np.float32)
    ln2_s = np.asarray(ln2_s, np.float32)
    ln2_b = np.asarray(ln2_b, np.float32)

    # fold LN1 affine into qkv weights, LN2 into fc1; SCALE into Q
    w_eff = qkv_w * ln1_s[:, None, :]                       # [L, 3D, D]
    b_eff = np.asarray(qkv_b, np.float32) + np.einsum('led,ld->le', qkv_w, ln1_b)
    w_eff[:, :D, :] *= SCALE
    b_eff[:, :D] *= SCALE

    f1_eff = fc1_w * ln2_s[:, None, :]                      # [L, MLP, D]
    bf1_eff = np.asarray(fc1_b, np.float32) + np.einsum('lmd,ld->lm', fc1_w, ln2_b)

    # Q/K: stationary-weight layout [L, mc, ki, kc*mi], m reordered to the
    # consumption order (Q0,K0,Q1,K1,...) then grouped 4-per-DMA
    wqkT = w_eff[:, :2 * D, :].transpose(0, 2, 1)           # [L, 768, 1536]
    wqk = (wqkT.reshape(L, KC, 128, 12, 128)
           .transpose(0, 3, 2, 1, 4).reshape(L, 12, 128, KC * 128))
    morder = [0, 6, 1, 7, 2, 8, 3, 9, 4, 10, 5, 11]
    wqk = (wqk[:, morder].reshape(L, 3, 4, 128, KC * 128)
           .transpose(0, 1, 3, 2, 4).reshape(L, 3, 128, 4 * KC * 128))
    # V / proj: moving-weight layout grouped 3 kc per DMA
    wv = w_eff[:, 2 * D:, :].transpose(0, 2, 1).reshape(L, KC, 128, D)
    wv = (wv.reshape(L, 2, 3, 128, D).transpose(0, 1, 3, 2, 4)
          .reshape(L, 2, 128, 3 * D))
    wp = np.asarray(proj_w, np.float32).transpose(0, 2, 1).reshape(L, KC, 128, D)
    wp = (wp.reshape(L, 2, 3, 128, D).transpose(0, 1, 3, 2, 4)
          .reshape(L, 2, 128, 3 * D))
    wf1T = f1_eff.transpose(0, 2, 1)                        # [L, 768, 3072]
    wf1 = (wf1T.reshape(L, KC, 128, 24, 128)
           .transpose(0, 3, 2, 1, 4).reshape(L, 24, 128, KC * 128))
    wf1 = (wf1.reshape(L, 6, 4, 128, KC * 128).transpose(0, 1, 3, 2, 4)
           .reshape(L, 6, 128, 4 * KC * 128))
    wf2 = np.asarray(fc2_w, np.float32).transpose(0, 2, 1).reshape(L, 24, 128, D)
    wf2 = (wf2.reshape(L, 6, 4, 128, D).transpose(0, 1, 3, 2, 4)
           .reshape(L, 6, 128, 4 * D))

    # patch embed
    wpe = (np.asarray(patch_w, np.float32).reshape(D, D).T
           .reshape(KC, 128, D))                            # [kc, ki, out]

    # bias0 (cls+pos folded), device row order: [orig 1..196, orig 0]
    pos = np.asarray(pos_embed, np.float32).reshape(S, D)
    b0 = pos.copy()
    b0[0] += np.asarray(cls_token, np.float32).reshape(D)
    b0[1:] += np.asarray(patch_b, np.float32)[None, :]
    bias0 = np.concatenate([b0[1:], b0[:1]], axis=0)        # [197, 768] dev order

    # binary attention mask, device layout (q rows: patches then CLS)
    routes = np.asarray(routes)
    mask = np.full((S, S), -30000.0, np.float32)            # additive log-mask
    mask[np.arange(P)[:, None], routes] = 0.0               # patch keys only
    mask[P, :] = 0.0                                        # CLS row: dense
    # per-partition biases for feature-major outputs [L, 128, m]
    bq = b_eff[:, :2 * D].reshape(L, 12, 128).transpose(0, 2, 1).copy()
    bf1 = bf1_eff.reshape(L, 24, 128).transpose(0, 2, 1).copy()

    shared = {
        "wqk": _bf16(wqk), "wv": _bf16(wv), "wp": _bf16(wp),
        "wf1": _bf16(wf1), "wf2": _bf16(wf2), "wpe": _bf16(wpe),
        "bias0": np.ascontiguousarray(bias0, dtype=np.float32),
        "maskb": _bf16(mask),
    }
    nz = {
        "bq": bool(np.abs(b_eff[:, :2 * D]).max() > 0),
        "bf1": bool(np.abs(bf1_eff).max() > 0),
        "bv": bool(np.abs(b_eff[:, 2 * D:]).max() > 0),
        "bp": bool(np.abs(np.asarray(proj_b, np.float32)).max() > 0),
        "bf2": bool(np.abs(np.asarray(fc2_b, np.float32)).max() > 0),
    }
    extras = {}
    if nz["bq"]:
        extras["bq"] = np.ascontiguousarray(bq, dtype=np.float32)
    if nz["bf1"]:
        extras["bf1"] = np.ascontiguousarray(bf1, dtype=np.float32)
    if nz["bv"]:
        extras["bvrow"] = np.ascontiguousarray(b_eff[:, 2 * D:], dtype=np.float32)
    if nz["bp"]:
        extras["bprow"] = np.ascontiguousarray(np.asarray(proj_b, np.float32))
    if nz["bf2"]:
        extras["bf2row"] = np.ascontiguousarray(np.asarray(fc2_b, np.float32))
    shared.update(extras)
    return shared, nz


def _prep_percore(x):
    """im2col + transpose: per-image xpT [kc, ki, 196] bf16."""
    x = np.asarray(x, np.float32)
    xp = (x.reshape(B, C, G, PATCH, G, PATCH)
          .transpose(0, 2, 4, 1, 3, 5).reshape(B, P, C * PATCH * PATCH))
    xpt = xp.transpose(0, 2, 1).reshape(B, KC, 128, P)      # [B, kc, ki, 196]
    return [_bf16(xpt[b]) for b in range(B)]


# ---------------------------------------------------------- device program

def _split_waits(nc, max_waits=1):
    """Walrus (this build) rejects >1 sem-wait on some instructions; hoist
    extra waits onto same-engine nops placed immediately before."""
    n = 0
    for f in nc.m.functions:
        for b in f.blocks:
            out = []
            for inst in list(b.instructions):
                si = inst.sync_info
                if si is not None and si.on_wait and len(si.on_wait) > max_waits:
                    waits = list(si.on_wait)
                    extra, keep = waits[:-max_waits], waits[-max_waits:]
                    while extra:
                        chunk, extra = extra[:max_waits], extra[max_waits:]
                        nop = mybir.InstNoOp(
                            name=nc.get_next_instruction_name(),
                            engine=inst.engine, bass_nofuse=True,
                            sync_info=mybir.SyncInfo(on_wait=chunk, on_update=[]),
                        )
                        nc.register_instruction(nop, overwrite=True)
                        out.append(nop)
                        n += 1
                    inst.sync_info = mybir.SyncInfo(
                        on_wait=keep, on_update=list(si.on_update or []))
                out.append(inst)
            b.instructions = out
    return n


def _build(nz_key):
    if nz_key in _BUILD_CACHE:
        return _BUILD_CACHE[nz_key]
    nz = dict(nz_key)

    nc = bass.Bass(num_devices=N_CORES)
    xpt_d = nc.dram_tensor("xpt", [KC, 128, P], BF16, kind="ExternalInput")
    wqk_d = nc.dram_tensor("wqk", [L, 3, 128, 4 * KC * 128], BF16,
                           kind="ExternalInput")
    wv_d = nc.dram_tensor("wv", [L, 2, 128, 3 * D], BF16, kind="ExternalInput")
    wp_d = nc.dram_tensor("wp", [L, 2, 128, 3 * D], BF16, kind="ExternalInput")
    wf1_d = nc.dram_tensor("wf1", [L, 6, 128, 4 * KC * 128], BF16,
                           kind="ExternalInput")
    wf2_d = nc.dram_tensor("wf2", [L, 6, 128, 4 * D], BF16,
                           kind="ExternalInput")
    wpe_d = nc.dram_tensor("wpe", [KC, 128, D], BF16, kind="ExternalInput")
    bias0_d = nc.dram_tensor("bias0", [S, D], F32, kind="ExternalInput")
    maskb_d = nc.dram_tensor("maskb", [S, S], BF16, kind="ExternalInput")
    if nz["bq"]:
        bq_d = nc.dram_tensor("bq", [L, 128, 12], F32, kind="ExternalInput")
    if nz["bf1"]:
        bf1_d = nc.dram_tensor("bf1", [L, 128, 24], F32, kind="ExternalInput")
    brow_d = {}
    for k in ("bv", "bp", "bf2"):
        if nz[k]:
            brow_d[k] = nc.dram_tensor(k + "row", [L, D], F32, kind="ExternalInput")
    out_d = nc.dram_tensor("out", [S, D], F32, kind="ExternalOutput")

    with tile.TileContext(nc) as tc, ExitStack() as ctx:
        const = ctx.enter_context(tc.tile_pool(name="const", bufs=1))
        xres = ctx.enter_context(tc.tile_pool(name="xres", bufs=1))
        act = ctx.enter_context(tc.tile_pool(name="act", bufs=2))
        sm = ctx.enter_context(tc.tile_pool(name="sm", bufs=3))
        mlp = ctx.enter_context(tc.tile_pool(name="mlp", bufs=6))
        wpool = ctx.enter_context(tc.tile_pool(name="wpool", bufs=3))
        psum = ctx.enter_context(tc.tile_pool(name="psum", bufs=1, space="PSUM"))

        ident = const.tile([128, 128], BF16)
        make_identity(nc, ident)
        eps_t = const.tile([128, 1], F32)
        nc.vector.memset(eps_t, EPS)
        # additive log-mask, token-major per q-chunk (applied on PE via
        # identity-matmul accumulate into the score psum)
        mask_sb = [const.tile([128, S], BF16, tag=f"mask{i}", name=f"mask{i}")
                   for i in range(2)]
        for i, (q0, qw) in enumerate(QCH):
            nc.sync.dma_start(out=mask_sb[i][:qw], in_=maskb_d[q0:q0 + qw])
        bias0_sb = [const.tile([128, D], F32, tag=f"b0{i}", name=f"b0{i}")
                    for i in range(2)]
        nc.sync.dma_start(out=bias0_sb[0], in_=bias0_d[0:T1])
        nc.sync.dma_start(out=bias0_sb[1][:T2], in_=bias0_d[T1:S])

        x_t = [xres.tile([128, D], F32, tag=f"x{i}", name=f"x{i}") for i in range(2)]

        def psum_big(j):
            return psum.tile([128, 512], F32, tag=f"big{j}",
                             name=f"big{j}", bufs=1)

        rot = [0]

        def psum_rot():
            # single-bank f32 [128,<=512] rotating over big0..3 (deep
            # pipelining for the qk / fc1 GEMM streams)
            j = rot[0] % 4
            rot[0] += 1
            return psum.tile([128, S], F32, tag=f"big{j}", name=f"big{j}",
                             bufs=1, padded_shape=[128, 512])

        def psum_s():
            return psum.tile([128, S], F32, tag="s", name="s", bufs=2,
                             padded_shape=[128, 512])

        def psum_tr():
            return psum.tile([128, 2, SPAD], BF16, tag="tr", name="tr", bufs=2,
                             padded_shape=[128, 2, 512])

        def psum_f32_tr():
            return psum.tile([128, S], F32, tag="tr", name="tr", bufs=2,
                             padded_shape=[128, 512])

        def bcast_add(x_tiles, row_ap):
            brow = sm.tile([128, D], F32, tag="brow", name="brow")
            bc = bass.AP(tensor=row_ap.tensor, offset=row_ap.offset,
                         ap=[[0, 128]] + list(row_ap.ap))
            nc.gpsimd.dma_start(out=brow, in_=bc)
            for i, (q0, qw) in enumerate(QCH):
                nc.vector.tensor_add(out=x_tiles[i][:qw], in0=x_tiles[i][:qw],
                                     in1=brow[:qw])

        # ---------------- patch embed: x = xpT.T @ wpe + bias0 ----------------
        pe_ps = {}
        for i in range(2):
            for j in range(2):
                pe_ps[(i, j)] = psum_big(2 * i + j)
        for kc in range(KC):
            xpt_sb = wpool.tile([128, P], BF16, tag="wxp", name="wxp")
            nc.sync.dma_start(out=xpt_sb, in_=xpt_d[kc])
            wpe_sb = wpool.tile([128, D], BF16, tag="wpe", name="wpe")
            nc.sync.dma_start(out=wpe_sb, in_=wpe_d[kc])
            for i, (q0, qw) in enumerate([(0, 128), (128, P - 128)]):
                for j, (n0, nw) in enumerate(NH):
                    nc.tensor.matmul(pe_ps[(i, j)][:qw, :nw],
                                     xpt_sb[:, q0:q0 + qw],
                                     wpe_sb[:, n0:n0 + nw],
                                     start=(kc == 0), stop=(kc == KC - 1))
        nc.vector.tensor_copy(out=x_t[1][:T2], in_=bias0_sb[1][:T2])
        for j, (n0, nw) in enumerate(NH):
            nc.vector.tensor_add(out=x_t[0][:, n0:n0 + nw],
                                 in0=pe_ps[(0, j)][:, :nw],
                                 in1=bias0_sb[0][:, n0:n0 + nw])
            nc.vector.tensor_add(out=x_t[1][:P - 128, n0:n0 + nw],
                                 in0=x_t[1][:P - 128, n0:n0 + nw],
                                 in1=pe_ps[(1, j)][:P - 128, :nw])

        # ---------------- transformer layers ----------------
        for l in range(L):
            if nz["bq"]:
                bqt = sm.tile([128, 12], F32, tag="bqt", name="bqt")
                nc.sync.dma_start(out=bqt, in_=bq_d[l])
            if nz["bf1"]:
                bf1t = sm.tile([128, 24], F32, tag="bf1t", name="bf1t")
                nc.sync.dma_start(out=bf1t, in_=bf1_d[l])

            def layernorm_chunk(i, tag):
                """x chunk (fp32 token-major) -> xn bf16 token-major tile."""
                q0, qw = QCH[i]
                st = sm.tile([128, 2, 6], F32, tag=f"st{tag}{i}",
                             name=f"st{tag}{i}")
                xg = x_t[i][:qw].rearrange("p (s f) -> p s f", s=2)
                for g in range(2):
                    nc.vector.bn_stats(out=st[:qw, g], in_=xg[:, g])
                mv = sm.tile([128, 2], F32, tag=f"mv{tag}{i}", name=f"mv{tag}{i}")
                nc.vector.bn_aggr(out=mv[:qw], in_=st[:qw])
                rs = sm.tile([128, 1], F32, tag=f"rs{tag}{i}", name=f"rs{tag}{i}")
                # rstd = exp(-0.5*ln(var+eps)): ln/exp share one ACT table
                # with the softmax exp, so the LN chain never swaps tables
                nc.scalar.activation(out=rs[:qw], in_=mv[:qw, 1:2],
                                     func=mybir.ActivationFunctionType.Ln,
                                     bias=eps_t[:qw])
                nc.scalar.activation(out=rs[:qw], in_=rs[:qw],
                                     func=mybir.ActivationFunctionType.Exp,
                                     scale=-0.5)
                xn_i = sm.tile([128, D], BF16, tag=f"xn{tag}{i}",
                               name=f"xn{tag}{i}")
                nc.vector.tensor_scalar(out=xn_i[:qw], in0=x_t[i][:qw],
                                        scalar1=mv[:qw, 0:1], scalar2=rs[:qw],
                                        op0=mybir.AluOpType.subtract,
                                        op1=mybir.AluOpType.mult)
                return xn_i

            def transpose_to_fm(xn):
                xnT = []
                for kc in range(KC):
                    pt = psum_tr()
                    nc.tensor.transpose(pt[:, 0, 0:T1],
                                        xn[0][:, kc * 128:(kc + 1) * 128], ident)
                    nc.tensor.transpose(pt[:, 0, T1:S],
                                        xn[1][:T2, kc * 128:(kc + 1) * 128],
                                        ident[:T2, :T2])
                    t = act.tile([128, S], BF16, tag=f"xnt{kc}", name=f"xnt{kc}")
                    nc.vector.tensor_copy(out=t, in_=pt[:, 0, :S])
                    xnT.append(t)
                return xnT

            _s = nc.enter_named_scope(f"L{l:02d}_ln1", False)
            xn = [layernorm_chunk(0, "a"), layernorm_chunk(1, "a")]
            xnT = transpose_to_fm(xn)
            nc.leave_named_scope(f"L{l:02d}_ln1", _s[0], False)

            # ---- attention phase A: Q/K GEMMs pipelined with per-pair
            #      softmax (mask added on PE, rowsum via exp accum_out) and
            #      P-transposes buffered in SBUF until AV
            _s = nc.enter_named_scope(f"L{l:02d}_qkattn", False)
            attnT = [act.tile([128, S], BF16, tag=f"at{kc}", name=f"at{kc}")
                     for kc in range(KC)]
            qk = {}
            pT_all = {}
            qk_c = 0
            for p in range(6):
                for m in (p, 6 + p):
                    if qk_c % 4 == 0:
                        wq4 = wpool.tile([128, 4, KC * 128], BF16, tag="wqk",
                                         name="wqk", bufs=2)
                        nc.sync.dma_start(out=wq4, in_=wqk_d[l, qk_c // 4])
                    wt = wq4[:, qk_c % 4]
                    qk_c += 1
                    pq = psum_rot()
                    for kc in range(KC):
                        nc.tensor.matmul(pq[:, :S],
                                         wt[:, kc * 128:(kc + 1) * 128],
                                         xnT[kc], start=(kc == 0),
                                         stop=(kc == KC - 1))
                    t = act.tile([128, S], BF16, tag=f"qk{m}", name=f"qk{m}")
                    if nz["bq"]:
                        nc.scalar.activation(
                            out=t, in_=pq[:, :S],
                            func=mybir.ActivationFunctionType.Identity,
                            bias=bqt[:, m:m + 1])
                    else:
                        nc.vector.tensor_copy(out=t, in_=pq[:, :S])
                    qk[m] = t

                # two heads h0=2p, h1=2p+1
                sc = []
                for i, (q0, qw) in enumerate(QCH):
                    pe2 = sm.tile([128, 2, SPAD], BF16, tag=f"p{i}", name=f"p{i}")
                    rsum = sm.tile([128, 2], F32, tag=f"rsum{i}", name=f"rsum{i}")
                    for j in range(2):
                        o = j * 64
                        ps = psum_s()
                        nc.tensor.matmul(ps[:qw],
                                         qk[p][o:o + 64, q0:q0 + qw],
                                         qk[6 + p][o:o + 64, :],
                                         start=True, stop=False)
                        nc.tensor.matmul(ps[:qw], ident[:qw, :qw],
                                         mask_sb[i][:qw],
                                         start=False, stop=True)
                        nc.scalar.activation(
                            out=pe2[:qw, j, :S], in_=ps[:qw],
                            func=mybir.ActivationFunctionType.Exp,
                            accum_out=rsum[:qw, j:j + 1])
                    nc.vector.reciprocal(out=rsum[:qw], in_=rsum[:qw])
                    for j in range(2):
                        nc.vector.tensor_scalar_mul(out=pe2[:qw, j, :S],
                                                    in0=pe2[:qw, j, :S],
                                                    scalar1=rsum[:qw, j:j + 1])
                    sc.append(pe2)
                # transpose P (both heads) -> keys-on-partitions tiles
                for kci, (k0, kw) in enumerate(QCH):
                    pt = psum_tr()
                    for j in range(2):
                        nc.tensor.transpose(pt[:kw, j, 0:T1],
                                            sc[0][:, j, k0:k0 + kw], ident)
                        nc.tensor.transpose(pt[:kw, j, T1:S],
                                            sc[1][:T2, j, k0:k0 + kw],
                                            ident[:T2, :T2])
                    t = sm.tile([128, 2, SPAD], BF16, tag=f"pt{p}_{kci}",
                                name=f"pt{p}_{kci}", bufs=1)
                    nc.vector.tensor_copy(out=t[:kw, :, :S],
                                          in_=pt[:kw, :, :S])
                    pT_all[(p, kci)] = t

            nc.leave_named_scope(f"L{l:02d}_qkattn", _s[0], False)
            _s = nc.enter_named_scope(f"L{l:02d}_v", False)
            # ---- attention phase B: V token-major GEMM
            v_ps = {}
            for i in range(2):
                for j in range(2):
                    v_ps[(i, j)] = psum_big(2 * i + j)
            for kc in range(KC):
                if kc % 3 == 0:
                    wt3 = wpool.tile([128, 3, D], BF16, tag="wv", name="wv",
                                     bufs=2)
                    nc.sync.dma_start(out=wt3, in_=wv_d[l, kc // 3])
                wt = wt3[:, kc % 3]
                for i, (q0, qw) in enumerate(QCH):
                    for j, (n0, nw) in enumerate(NH):
                        nc.tensor.matmul(v_ps[(i, j)][:qw, :nw],
                                         xnT[kc][:, q0:q0 + qw],
                                         wt[:, n0:n0 + nw],
                                         start=(kc == 0), stop=(kc == KC - 1))
            v_sb = []
            for i, (q0, qw) in enumerate(QCH):
                t = act.tile([128, D], BF16, tag=f"v{i}", name=f"v{i}")
                for j, (n0, nw) in enumerate(NH):
                    nc.vector.tensor_copy(out=t[:qw, n0:n0 + nw],
                                          in_=v_ps[(i, j)][:qw, :nw])
                v_sb.append(t)
            if nz["bv"]:
                brow = sm.tile([128, D], F32, tag="brow", name="brow")
                r = brow_d["bv"][l]
                nc.gpsimd.dma_start(out=brow, in_=bass.AP(
                    tensor=r.tensor, offset=r.offset, ap=[[0, 128]] + list(r.ap)))
                for i, (q0, qw) in enumerate(QCH):
                    nc.vector.tensor_add(out=v_sb[i][:qw], in0=v_sb[i][:qw],
                                         in1=brow[:qw])

            nc.leave_named_scope(f"L{l:02d}_v", _s[0], False)
            _s = nc.enter_named_scope(f"L{l:02d}_av", False)
            # ---- attention phase C: AV for all heads
            for p in range(6):
                for j in range(2):
                    h = 2 * p + j
                    pav = psum_s() if j == 0 else psum_f32_tr()
                    for kci, (k0, kw) in enumerate(QCH):
                        nc.tensor.matmul(pav[:64],
                                         v_sb[kci][:kw, h * 64:(h + 1) * 64],
                                         pT_all[(p, kci)][:kw, j, :S],
                                         start=(kci == 0), stop=(kci == 1))
                    nc.vector.tensor_copy(out=attnT[p][j * 64:(j + 1) * 64, :],
                                          in_=pav[:64])

            nc.leave_named_scope(f"L{l:02d}_av", _s[0], False)
            _s = nc.enter_named_scope(f"L{l:02d}_proj", False)
            # ---- proj token-major GEMM + residual
            pr_ps = {}
            for i in range(2):
                for j in range(2):
                    pr_ps[(i, j)] = psum_big(2 * i + j)
            for kc in range(KC):
                if kc % 3 == 0:
                    wp3 = wpool.tile([128, 3, D], BF16, tag="wp", name="wp",
                                     bufs=2)
                    nc.sync.dma_start(out=wp3, in_=wp_d[l, kc // 3])
                wt = wp3[:, kc % 3]
                for i, (q0, qw) in enumerate(QCH):
                    for j, (n0, nw) in enumerate(NH):
                        nc.tensor.matmul(pr_ps[(i, j)][:qw, :nw],
                                         attnT[kc][:, q0:q0 + qw],
                                         wt[:, n0:n0 + nw],
                                         start=(kc == 0), stop=(kc == KC - 1))
            for i, (q0, qw) in enumerate(QCH):
                for j, (n0, nw) in enumerate(NH):
                    nc.vector.tensor_add(out=x_t[i][:qw, n0:n0 + nw],
                                         in0=x_t[i][:qw, n0:n0 + nw],
                                         in1=pr_ps[(i, j)][:qw, :nw])
            if nz["bp"]:
                bcast_add(x_t, brow_d["bp"][l])

            nc.leave_named_scope(f"L{l:02d}_proj", _s[0], False)
            # ---- LN2 + transpose
            _s = nc.enter_named_scope(f"L{l:02d}_ln2", False)
            xn2 = [layernorm_chunk(0, "b"), layernorm_chunk(1, "b")]
            xn2T = transpose_to_fm(xn2)
            nc.leave_named_scope(f"L{l:02d}_ln2", _s[0], False)
            _s = nc.enter_named_scope(f"L{l:02d}_mlp", False)

            # ---- MLP: fc1 (feature-major) + gelu, fc2 (token-major) + residual
            f2_ps = {}
            for i in range(2):
                for j in range(2):
                    f2_ps[(i, j)] = psum_big(2 * i + j)
            for m in range(24):
                if m % 4 == 0:
                    wf4 = wpool.tile([128, 4, KC * 128], BF16, tag="wf1",
                                     name="wf1", bufs=2)
                    nc.sync.dma_start(out=wf4, in_=wf1_d[l, m // 4])
                wt = wf4[:, m % 4]
                ph = psum_s() if m % 2 == 0 else psum_f32_tr()
                for kc in range(KC):
                    nc.tensor.matmul(ph, wt[:, kc * 128:(kc + 1) * 128],
                                     xn2T[kc], start=(kc == 0),
                                     stop=(kc == KC - 1))
                hf = mlp.tile([128, S], BF16, tag="hfm", name="hfm")
                if nz["bf1"]:
                    nc.scalar.activation(out=hf, in_=ph,
                                         func=mybir.ActivationFunctionType.Gelu,
                                         bias=bf1t[:, m:m + 1])
                else:
                    nc.scalar.activation(out=hf, in_=ph,
                                         func=mybir.ActivationFunctionType.Gelu)
                if m % 4 == 0:
                    wg4 = wpool.tile([128, 4, D], BF16, tag="wf2",
                                     name="wf2", bufs=2)
                    nc.sync.dma_start(out=wg4, in_=wf2_d[l, m // 4])
                wt2 = wg4[:, m % 4]
                for i, (q0, qw) in enumerate(QCH):
                    for j, (n0, nw) in enumerate(NH):
                        nc.tensor.matmul(f2_ps[(i, j)][:qw, :nw],
                                         hf[:, q0:q0 + qw], wt2[:, n0:n0 + nw],
                                         start=(m == 0), stop=(m == 23))
            for i, (q0, qw) in enumerate(QCH):
                for j, (n0, nw) in enumerate(NH):
                    nc.vector.tensor_add(out=x_t[i][:qw, n0:n0 + nw],
                                         in0=x_t[i][:qw, n0:n0 + nw],
                                         in1=f2_ps[(i, j)][:qw, :nw])
            if nz["bf2"]:
                bcast_add(x_t, brow_d["bf2"][l])
            nc.leave_named_scope(f"L{l:02d}_mlp", _s[0], False)

        # ---------------- output ----------------
        nc.sync.dma_start(out=out_d[0:T1], in_=x_t[0])
        nc.sync.dma_start(out=out_d[T1:S], in_=x_t[1][:T2])

    _split_waits(nc)

    _BUILD_CACHE[nz_key] = nc
    return nc


# ---------------------------------------------------------------- entry

def run(inputs, trace=False):
    """Returns (full_output [B,S,D] fp32, BassKernelResults)."""
    x = inputs["x"]
    shared, nz = _prep_shared(
        inputs["patch_w"], inputs["patch_b"], inputs["cls_token"],
        inputs["pos_embed"], inputs["qkv_w"], inputs["qkv_b"],
        inputs["proj_w"], inputs["proj_b"], inputs["ln1_s"], inputs["ln1_b"],
        inputs["ln2_s"], inputs["ln2_b"], inputs["fc1_w"], inputs["fc1_b"],
        inputs["fc2_w"], inputs["fc2_b"], inputs["routes"])
    xpts = _prep_percore(x)
    nz_key = tuple(sorted(nz.items()))
    nc = _build(nz_key)

    in_maps = [dict(shared, xpt=xpts[c]) for c in range(N_CORES)]
    res = run_bass_kernel_spmd(nc, in_maps, core_ids=list(range(N_CORES)),
                               trace=trace)
    outs = []
    for c in range(N_CORES):
        dev = res.results[c]["out"]                          # [197, 768] dev order
        outs.append(np.concatenate([dev[P:S], dev[0:P]], axis=0))
    return np.stack(outs, axis=0).astype(np.float32), res


def kernel(**inputs):
    out, _ = run(inputs, trace=False)
    return out

